# revision 50
# baseline (speedup 1.0000x reference)
"""Multi-head causal self-attention (B=256, T=256, C=384, H=6, D=64) on 8
Trainium2 NeuronCores, data-parallel over the batch dimension (32 batches per
core, no collectives).

Per-core dataflow (fp8 Q/K projections via DoubleRow, bf16 elsewhere, fp32
PSUM accumulation):
  - Q/K projections run in fp8e4 with perf_mode=DoubleRow over two of the
    three contraction chunks (weights scaled 2^6 each on the host to clear
    the fp8 denormal floor; the 2^12 is divided out in the exp scale).
    They produce transposed activations Qt/Kt [e, t] so the score matmul
    contracts head dims on partitions; V stays bf16 [t, e] (fp8 V or output
    projections would breach the 2e-2 error gate - measured).
  - Scores are computed transposed, St[j, i] (keys on partitions), in a
    causal-compacted [P, 384] layout per head (the fully masked
    keys-hi x queries-lo quarter is never computed), so exp(St) feeds the
    P@V matmul directly with no on-chip transposes.  The 0/1 triangle mask
    is applied multiplicatively on the otherwise idle GpSimd engine.
  - Softmax denominators come from a ones-column matmul over exp(St) into
    the same PSUM bank as the P@V output (O in cols 0:256, denominators in
    256:512); the matmul itself replicates each head's denominator across
    its 64 partitions (engines cannot partition-broadcast from SBUF).
  - The per-query reciprocal is exp(-ln(den)) on the scalar engine: Ln and
    Exp share one activation table set, where a Reciprocal activation would
    force two ~2.7us ACT_TABLE_LOAD swaps per batch.
  - The batch loop is software-pipelined: batch b's attention output (P@V +
    normalize + output projection, consuming exps that finished during the
    previous step) is emitted before batch b+1's scores, keeping the
    per-engine FIFOs free of cross-engine stalls.  Score matmuls chain
    row-group concurrency and P@V/denominator matmuls col-group concurrency
    on the PE array, so the two phases are never interleaved.
  - Group g+1's 12 Q/K projection tiles are emitted 4-per-batch interleaved
    after each of group g's per-batch score phases (group 0 up front), so
    the projection matmuls fill PE bubbles where attention chains wait on
    the scalar engine, instead of sitting head-of-line blocked on their own
    PSUM drains in an up-front block (the PE FIFO runs matmuls strictly in
    emission order).
  - PSUM budget is exactly 8 banks: 3 io (QK proj / V proj / out proj
    rotation), 3 score tiles, 2 P@V banks.  Hard-won tuning notes: any
    bufs=1 structure (or moving banks between these pools) serializes a
    pipeline joint, starves the PE, and re-triggers HAM clock oscillation
    (the PE drops to 1.2 GHz in 6.8us windows - measured +45% wall); the
    scalar engine cannot absorb any PSUM drains on top of its exp/recip
    load; custom-DVE/ISA ops (reciprocal_approx_fast, tensor_mask_reduce)
    fail this container's walrus, and the plain DVE RECIPROCAL runs at
    ~6 cyc/elem - hence the Ln/Exp reciprocal, with Ln running in place
    over the dead denominator region in PSUM (scalar's faster port).
"""

import numpy as np

import concourse.bass as bass
import concourse.tile as tile
from concourse import mybir
from concourse.bass_utils import run_bass_kernel_spmd

P = 128
B, T, C = 256, 256, 384
H, D = 6, 64
NCORES = 8
BL = B // NCORES  # 32 batches per core
G = 4  # batch group for Q/K projection weight reuse
F32 = mybir.dt.float32
F32R = mybir.dt.float32r
BF16 = mybir.dt.bfloat16
F8 = mybir.dt.float8e4
# Q/K weights are scaled by 2^6 each on the host so their ~0.02-sigma values
# clear the fp8e4 denormal floor; the combined 2^12 is divided back out in
# the exp activation's scale argument.
QK_SCALE_LOG2 = 6


def _split_drain_waits(nc, cap=1):
    """This container's walrus rejects instructions carrying more than one
    sync wait ("Too many sync wait commands"); hoist extras onto no-ops
    inserted before (same engine => executed in order)."""
    n_new = 0
    for f in nc.m.functions:
        for bb in f.blocks:
            il = bb.instructions
            out = []
            changed = False
            for inst in list(il):
                si = getattr(inst, "sync_info", None)
                if si is not None and len(si.on_wait) > cap:
                    waits = list(si.on_wait)
                    extra, keep = waits[:-cap], waits[-cap:]
                    for i in range(0, len(extra), cap):
                        nop = mybir.InstNoOp(
                            name=f"I-waitsplit-{n_new}",
                            sync_info=mybir.SyncInfo(
                                on_wait=extra[i : i + cap], on_update=[]
                            ),
                            bass_nofuse=True,
                            engine=inst.engine,
                        )
                        n_new += 1
                        out.append(nop)
                    si.on_wait = keep
                    changed = True
                out.append(inst)
            if changed:
                il.clear()
                il.extend(out)
    return n_new


def build_module(split_waits=True):
    nc = bass.Bass("TRN2", target_bir_lowering=False, debug=False)

    # inputs arrive pre-arranged host-side in the on-chip layout so every
    # DMA is contiguous per partition line (the strided rearrange loads cost
    # ~3x in descriptor overhead)
    xt_d = nc.dram_tensor("xt", [P, 3, BL, T], BF16, kind="ExternalInput").ap()
    xq_d = nc.dram_tensor("xq", [P, 3, BL, T], F8, kind="ExternalInput").ap()
    wq_d = nc.dram_tensor("wq", [P, 3, C], F8, kind="ExternalInput").ap()
    wk_d = nc.dram_tensor("wk", [P, 3, C], F8, kind="ExternalInput").ap()
    wv_d = nc.dram_tensor("wv", [P, 3, C], BF16, kind="ExternalInput").ap()
    wp_d = nc.dram_tensor("wp", [P, 3, C], BF16, kind="ExternalInput").ap()
    bp_d = nc.dram_tensor("bp", [C], F32R, kind="ExternalInput").ap()
    mask_d = nc.dram_tensor("mask", [P, P], BF16, kind="ExternalInput").ap()
    y_d = nc.dram_tensor("y", [BL, T, C], F32, kind="ExternalOutput").ap()

    with tile.TileContext(nc) as tc:
        with (
            tc.tile_pool(name="consts", bufs=1) as consts,
            tc.tile_pool(name="xg", bufs=2) as xg_pool,
            tc.tile_pool(name="xq", bufs=2) as xq_pool,
            tc.tile_pool(name="qt", bufs=4) as qt_pool,
            tc.tile_pool(name="kt", bufs=4) as kt_pool,
            tc.tile_pool(name="vsb", bufs=G + 2) as v_pool,
            tc.tile_pool(name="sts", bufs=8) as sts_pool,
            tc.tile_pool(name="ot", bufs=3) as ot_pool,
            tc.tile_pool(name="ysb", bufs=3) as y_pool,
            tc.tile_pool(name="rsb", bufs=2) as r_pool,
            tc.tile_pool(name="psio", bufs=3, space="PSUM") as ps_io,
            tc.tile_pool(name="psst", bufs=3, space="PSUM") as ps_st,
            tc.tile_pool(name="psorb", bufs=2, space="PSUM") as ps_orb,
        ):
            # ---- constants (compute-critical loads first: the first QK
            # projection only needs x and wq/wk; bias/mask/ones setup is not
            # needed until the first attention phase ~10us later) ----
            xt_r = xt_d
            xq_r = xq_d
            # startup order: the first QK projection needs only wq/wk (fp8,
            # half the bytes) and xq0; everything else arrives while the
            # first projections run
            wq_sb = consts.tile([P, 3, C], F8)
            wk_sb = consts.tile([P, 3, C], F8)
            wv_sb = consts.tile([P, 3, C], BF16)
            wp_sb = consts.tile([P, 3, C], BF16)
            # startup loads ordered by first use: the first Q-projection
            # job needs only wq + the first xq half, so those two
            # descriptors go first (~600ns each on the sync queue)
            xq0 = xq_pool.tile([P, 3, G, T], F8, tag="xq")
            xg0 = xg_pool.tile([P, 3, G, T], BF16, tag="xg")
            h = G // 2
            nc.sync.dma_start(wq_sb[:], wq_d[:])
            nc.sync.dma_start(xq0[:, :, 0:h, :], xq_r[:, :, 0:h, :])
            nc.sync.dma_start(wk_sb[:], wk_d[:])
            nc.sync.dma_start(xq0[:, :, h:G, :], xq_r[:, :, h:G, :])
            nc.sync.dma_start(xg0[:, :, 0:h, :], xt_r[:, :, 0:h, :])
            nc.sync.dma_start(xg0[:, :, h:G, :], xt_r[:, :, h:G, :])
            nc.sync.dma_start(wv_sb[:], wv_d[:])
            nc.sync.dma_start(wp_sb[:], wp_d[:])
            # (Note: a HAM-warmup matmul chain during the startup DMA wait
            # was tried and measured net-negative - the cold-clock warm
            # chain delays the first real projection by about what the
            # clock ramp costs, and mistiming it is expensive.)

            # partition-replication is done with rank-1 matmuls (ones ⊗ row):
            # step-0 partition-broadcast DMAs produce garbage on hardware.
            ones_row = consts.tile([1, P], F32)
            nc.vector.memset(ones_row[:], 1.0)
            ones_row_r = consts.tile([1, P], F32R)
            nc.scalar.activation(
                ones_row_r[:], ones_row[:], mybir.ActivationFunctionType.Copy
            )
            bp_row = consts.tile([1, C], F32R)
            nc.sync.dma_start(bp_row[:], bp_d[None, :])
            bp_sb = consts.tile([P, C], F32)
            mask_sb = consts.tile([P, P], BF16)
            nc.sync.dma_start(mask_sb[:], mask_d[:])

            ones_mat = consts.tile([P, P], BF16)
            nc.vector.memset(ones_mat[:], 1.0)

            def scores_pair(hp, qt, kt, st_pairs):
                """Stage 1, one head pair: score matmuls + exp + causal mask.

                Causal-compacted layout: st_pair [P, h, 384] where free cols
                0:256 hold (keys jt0) x (queries 0:256) and cols 256:384 hold
                (keys jt1) x (queries 128:256).  The fully masked
                (jt1, i<128) quarter is never computed.

                Per-head PSUM tiles with bufs=2 keep two heads' score
                matmuls in flight while the previous head's exp drains.
                """
                st_pair = sts_pool.tile([P, 2, 384], BF16, tag="stp")
                for hidx in range(2):
                    h = 2 * hp + hidx
                    co, half = h // 2, h % 2
                    st_ps = ps_st.tile([P, 384], F32, tag="st")
                    nc.tensor.matmul(
                        st_ps[:, 0:T],
                        (kt[64 * half : 64 * half + 64, co, 0:P]),
                        (qt[64 * half : 64 * half + 64, co, :]),
                        start=True,
                        stop=True,
                    )
                    nc.tensor.matmul(
                        st_ps[:, T : T + P],
                        (kt[64 * half : 64 * half + 64, co, P : 2 * P]),
                        (qt[64 * half : 64 * half + 64, co, P : 2 * P]),
                        start=True,
                        stop=True,
                    )
                    # exp the whole tile unmasked (scores are bounded, so
                    # exp never overflows); the causal mask is applied
                    # multiplicatively below.  scale undoes the 2^12 fp8
                    # weight scaling baked into qt/kt.
                    nc.scalar.activation(
                        st_pair[:, hidx, :],
                        st_ps[:],
                        mybir.ActivationFunctionType.Exp,
                        scale=2.0 ** (-2 * QK_SCALE_LOG2),
                    )
                    # cols 0:128 (jt0, i<128) and 256:384 (jt1, i>=128) are
                    # the same [128,128] 0/1 triangle.  Masked per head (not
                    # per pair) so each mask starts right after its own exp:
                    # the pair's last mask finishes ~530ns earlier, which is
                    # what gates the next pipeline step's P@V.
                    diag_s = st_pair.rearrange("p h (a c) -> p h a c", c=P)[
                        :, hidx : hidx + 1, 0::2, :
                    ]
                    nc.gpsimd.tensor_mul(
                        diag_s,
                        diag_s,
                        mask_sb[:, None, None, :].to_broadcast((P, 1, 2, P)),
                    )
                st_pairs.append(st_pair)

            def attn_pair(hp, st_pair, v_sb, ot):
                """Stage 2, one head pair: P@V and ones-matmul denominators
                (replicated across each head's 64 partitions) into one
                shared PSUM bank (O in cols 0:256, denominators in
                256:512), then the per-pair Ln/Exp reciprocal chain (the
                two share one ACT table set) and the normalize multiply.
                Emission alternates col groups for immediate dual-chain
                feed."""
                h0, h1 = 2 * hp, 2 * hp + 1
                orb = ps_orb.tile([P, 512], F32, tag="orb")
                # even head -> partitions 0:64, odd head -> 64:128 (bf16
                # col tile_position).  Queries below 128 only see jt0 keys,
                # so the jt1 matmul covers N=128 (cols 128:256).
                for jt in range(2):
                    for idx, h in enumerate((h0, h1)):
                        base = 64 * idx
                        if jt == 0:
                            nc.tensor.matmul(
                                orb[base : base + 64, 0:T],
                                (v_sb[:, 0, h, :]),
                                (st_pair[:, idx, 0:T]),
                                start=True,
                                stop=False,
                                tile_position=(0, base),
                            )
                        else:
                            nc.tensor.matmul(
                                orb[base : base + 64, P:T],
                                (v_sb[:, 1, h, :]),
                                (st_pair[:, idx, T : T + P]),
                                start=False,
                                stop=True,
                                tile_position=(0, base),
                            )
                for jt in range(2):
                    for idx in range(2):
                        base = 64 * idx
                        if jt == 0:
                            nc.tensor.matmul(
                                orb[base : base + 64, T : 2 * T],
                                (ones_mat[:, base : base + 64]),
                                (st_pair[:, idx, 0:T]),
                                start=True,
                                stop=False,
                                tile_position=(0, base),
                            )
                        else:
                            nc.tensor.matmul(
                                orb[base : base + 64, T + P : 2 * T],
                                (ones_mat[:, base : base + 64]),
                                (st_pair[:, idx, T : T + P]),
                                start=False,
                                stop=True,
                                tile_position=(0, base),
                            )
                # Ln in place over the denominator region (the dens are
                # dead after this), then Exp(-x) reading PSUM: the scalar
                # engine's PSUM ports are faster than SBUF (errata: SBUF-src
                # 224+FD cyc vs PSUM-src 172+FD), and elementwise in-place
                # is pipeline-safe (out[i] writes ~8 cycles after in[i] is
                # read).
                nc.scalar.activation(
                    orb[:, T : 2 * T],
                    orb[:, T : 2 * T],
                    mybir.ActivationFunctionType.Ln,
                )
                rb = r_pool.tile([P, T], F32, tag="rb")
                nc.scalar.activation(
                    rb[:],
                    orb[:, T : 2 * T],
                    mybir.ActivationFunctionType.Exp,
                    scale=-1.0,
                )
                nc.vector.tensor_mul(ot[:, hp, :], orb[:, 0:T], rb[:])

            def out_proj(b, ot):
                """Stage 2 tail: output projection + bias + store."""
                for tt in range(2):
                    ps = ps_io.tile([P, C], F32, tag="io")
                    for co in range(3):
                        nc.tensor.matmul(
                            ps[:],
                            (ot[:, co, tt * P : (tt + 1) * P]),
                            (wp_sb[:, co, :]),
                            start=(co == 0),
                            stop=(co == 2),
                        )
                    y_sb = y_pool.tile([P, C], F32)
                    nc.vector.tensor_add(y_sb[:], ps[:], bp_sb[:])
                    nc.sync.dma_start(y_d[b, tt * P : (tt + 1) * P, :], y_sb[:])

            # Software pipeline: batch b's scores (tensor) run while batch
            # b-1's exp results feed its P@V (so the tensor engine never
            # waits on the scalar engine's exps), interleaved pair-by-pair.
            prev = None  # (b, st_pairs, v_sb)
            def qk_tiles_alloc(g):
                qt2s = [
                    qt_pool.tile([P, 3, 2, T], BF16, tag="qtb", name=f"qtb_{g}_{bp2}")
                    for bp2 in range(G // 2)
                ]
                kt2s = [
                    kt_pool.tile([P, 3, 2, T], BF16, tag="ktb", name=f"ktb_{g}_{bp2}")
                    for bp2 in range(G // 2)
                ]
                return qt2s, kt2s

            def qk_tile_jobs(xq_g, qt2s, kt2s):
                """One emission closure per Q/K projection output tile
                (proj, eo, bp2).  fp8: contraction chunks co 0..1 in one
                DoubleRow matmul (virtual 256-row array), chunk 2 as a plain
                fp8 matmul, then a drain PSUM->SBUF bf16."""

                def job(w_sb, dst_list, eo, bp2):
                    ps = ps_io.tile([P, 512], F32, tag="io")
                    rhs = xq_g[:, :, 2 * bp2 : 2 * bp2 + 2, :].rearrange(
                        "p c b t -> p c (b t)"
                    )
                    nc.tensor.matmul(
                        ps[:],
                        (w_sb[:, 0:2, eo * P : (eo + 1) * P]),
                        (rhs[:, 0:2, :]),
                        start=True,
                        stop=False,
                        perf_mode=mybir.MatmulPerfMode.DoubleRow,
                    )
                    nc.tensor.matmul(
                        ps[:],
                        (w_sb[:, 2, eo * P : (eo + 1) * P]),
                        (rhs[:, 2, :]),
                        start=False,
                        stop=True,
                    )
                    dst_ap = dst_list[bp2][:, eo, :, :].rearrange("p b t -> p (b t)")
                    # all drains on vector: the scalar engine is saturated
                    # by exps + the reciprocal chain (measured: giving it
                    # even 1-in-3 drains pushes it to 212us busy and the
                    # resulting exp delays re-trigger HAM oscillation)
                    nc.vector.tensor_copy(dst_ap, ps[:])

                from functools import partial

                return [
                    partial(job, w_sb, dst_list, eo, bp2)
                    for bp2 in range(G // 2)
                    for w_sb, dst_list in ((wq_sb, qt2s), (wk_sb, kt2s))
                    for eo in range(3)
                ]

            # Group 0's Q/K tiles are computed up front (startup); from then
            # on group g+1's 12 projection tiles are emitted 4-per-batch
            # interleaved AFTER each batch's score matmuls, so they fill the
            # PE bubbles where score/attention chains wait on the scalar
            # engine's exps - instead of v1's up-front block, whose matmuls
            # sat head-of-line blocked on their own PSUM drains (the PE FIFO
            # executes matmuls strictly in emission order).
            cur_x = (xg0, xq0)
            cur_qk = qk_tiles_alloc(0)
            for j in qk_tile_jobs(xq0, *cur_qk):
                j()
            nxt_x = None
            nxt_qk = None
            jobs = []
            carry = []
            for g in range(BL // G):
                xg, xq = cur_x
                qt2s, kt2s = cur_qk

                if g == 0:
                    # bias broadcast, deferred so the first QK projections
                    # are not stuck behind the bp DMA chain at startup
                    bp_ps = ps_io.tile([P, C], F32, tag="io")
                    nc.tensor.matmul(
                        bp_ps[:], ones_row_r[0:1, :], bp_row[0:1, :],
                        start=True, stop=True,
                    )
                    nc.vector.tensor_copy(bp_sb[:], bp_ps[:])

                for lb in range(G):
                    b = g * G + lb
                    if lb == 0 and g + 1 < BL // G:
                        # prefetch next group's x now; its Q/K projection
                        # jobs start at lb=1, a full batch (~7us) after the
                        # DMA is issued
                        gn = (g + 1) * G
                        h = G // 2
                        xqn = xq_pool.tile([P, 3, G, T], F8, tag="xq", name=f"xq_{g+1}")
                        nc.sync.dma_start(xqn[:, :, 0:h, :], xq_r[:, :, gn : gn + h, :])
                        nc.sync.dma_start(xqn[:, :, h:G, :], xq_r[:, :, gn + h : gn + G, :])
                        xgn = xg_pool.tile([P, 3, G, T], BF16, tag="xg", name=f"xg_{g+1}")
                        nc.sync.dma_start(xgn[:, :, 0:h, :], xt_r[:, :, gn : gn + h, :])
                        nc.sync.dma_start(xgn[:, :, h:G, :], xt_r[:, :, gn + h : gn + G, :])
                        nxt_x = (xgn, xqn)
                        nxt_qk = qk_tiles_alloc(g + 1)
                        jobs = qk_tile_jobs(xqn, *nxt_qk)
                    qt = qt2s[lb // 2][:, :, lb % 2, :]
                    kt = kt2s[lb // 2][:, :, lb % 2, :]

                    # ---- V projection: V[t, e] (x stationary) ----
                    v_sb = v_pool.tile([P, 2, H, D], BF16)
                    for tt in range(2):
                        ps = ps_io.tile([P, C], F32, tag="io")
                        for co in range(3):
                            nc.tensor.matmul(
                                ps[:],
                                (xg[:, co, lb, tt * P : (tt + 1) * P]),
                                (wv_sb[:, co, :]),
                                start=(co == 0),
                                stop=(co == 2),
                            )
                        nc.vector.tensor_copy(
                            v_sb[:, tt, :, :].rearrange("p h d -> p (h d)"), ps[:]
                        )

                    # ---- attention-out[b-1], then scores[b] ----
                    # Phase-grouped: the 12 score matmuls chain row-group
                    # concurrency and the 12 P@V/denominator matmuls chain
                    # col-group concurrency; a score and a P@V matmul can
                    # never overlap on the PE array (scores use all 128
                    # columns, P@V all 128 rows), so mixing them costs
                    # serialization joints.  Batch-level pipelining: batch
                    # b-1's attention output (whose exps finished during the
                    # previous step) runs first; batch b's score exps queue
                    # up behind it and complete during this step's tensor
                    # work.
                    if prev is not None:
                        ot_prev = ot_pool.tile([P, 3, T], BF16, tag="ot", name="ot_prev")
                        for hp in range(3):
                            attn_pair(hp, prev[1][hp], prev[2], ot_prev)
                        out_proj(prev[0], ot_prev)
                    st_pairs = []
                    for hp in range(3):
                        scores_pair(hp, qt, kt, st_pairs)
                    prev = (b, st_pairs, v_sb)
                    # Q/K projection tiles for the NEXT group, emitted 3 per
                    # batch in a window shifted half a group: the bp2=0
                    # tiles (needed first, by the next group's b0/b1 scores)
                    # during this group's lb=2,3, the bp2=1 tiles carried
                    # into the next group's lb=0,1.  This evens the filler
                    # load to 3/3/3/3 per batch (the uneven 0/4/4/4 left the
                    # group-boundary step bare, and concentrating 6/6/0
                    # saturated the vector drain queue - both measured
                    # worse) and, crucially, gives the LAST group's steps
                    # filler too, where ~1us P@V pipeline-drain gaps showed.
                    if lb <= 1 and carry:
                        for j in carry[3 * lb : 3 * lb + 3]:
                            j()
                    if lb >= 2 and jobs:
                        for j in jobs[3 * (lb - 2) : 3 * (lb - 1)]:
                            j()
                cur_x = nxt_x
                cur_qk = nxt_qk
                carry = jobs[6:12] if jobs else []
                jobs = []

            # ---- drain the last batch ----
            ot_prev = ot_pool.tile([P, 3, T], BF16, tag="ot", name="ot_last")
            for hp in range(3):
                attn_pair(hp, prev[1][hp], prev[2], ot_prev)
            out_proj(prev[0], ot_prev)

    if split_waits:
        _split_drain_waits(nc)
    return nc


_NC = None


def _get_nc():
    global _NC
    if _NC is None:
        _NC = build_module()
    return _NC


def make_mask():
    # [128, 128] 0/1 triangle block: key p visible to query i when p <= i
    import ml_dtypes

    j = np.arange(P)[:, None]
    i = np.arange(P)[None, :]
    return np.where(j <= i, 1.0, 0.0).astype(ml_dtypes.bfloat16)


def prepare_in_maps(x, Wk, Wq, Wv, Wp, bp):
    import ml_dtypes

    bf16 = ml_dtypes.bfloat16
    f8 = ml_dtypes.float8_e4m3
    xf = np.asarray(x, dtype=np.float32).transpose(2, 0, 1)
    xf = xf.reshape(3, P, *xf.shape[1:]).transpose(1, 0, 2, 3)  # [ci, co, b, t]
    xt = np.ascontiguousarray(xf.astype(bf16))
    xq = np.ascontiguousarray(xf.astype(f8))
    # 1/sqrt(D) folded into Wq (exact exponent shift); both Q/K weights are
    # scaled by 2^6 so their values clear the fp8e4 denormal floor (the 2^12
    # total is divided back out in the kernel's exp scale argument)
    s8 = float(2**QK_SCALE_LOG2)

    def chipw(w):  # [C, C] -> [ci, co, e] contiguous
        return np.ascontiguousarray(w.reshape(3, P, C).transpose(1, 0, 2))

    wq = chipw((np.asarray(Wq, dtype=np.float32).T * (0.125 * s8)).astype(f8))
    wk = chipw((np.asarray(Wk, dtype=np.float32).T * s8).astype(f8))
    wv = chipw(np.asarray(Wv, dtype=np.float32).T.astype(bf16))
    wp = chipw(np.asarray(Wp, dtype=np.float32).T.astype(bf16))
    bp = np.asarray(bp, dtype=np.float32)
    mask = make_mask()
    in_maps = []
    for c in range(NCORES):
        in_maps.append(
            {
                "xt": np.ascontiguousarray(xt[:, :, c * BL : (c + 1) * BL, :]),
                "xq": np.ascontiguousarray(xq[:, :, c * BL : (c + 1) * BL, :]),
                "wq": wq,
                "wk": wk,
                "wv": wv,
                "wp": wp,
                "bp": bp,
                "mask": mask,
            }
        )
    return in_maps


def kernel(x, Wk, Wq, Wv, Wp, bp):
    nc = _get_nc()
    in_maps = prepare_in_maps(x, Wk, Wq, Wv, Wp, bp)
    res = run_bass_kernel_spmd(nc, in_maps, list(range(NCORES)))
    return np.concatenate([r["y"] for r in res.results], axis=0)



# revision 51
# speedup vs baseline: 1.1148x; 1.1148x over previous
"""Multi-head causal self-attention (B=256, T=256, C=384, H=6, D=64) on 8
Trainium2 NeuronCores, data-parallel over the batch dimension (32 batches per
core, no collectives).

Per-core dataflow (fp8 Q/K projections via DoubleRow, bf16 elsewhere, fp32
PSUM accumulation):
  - Q/K projections run in fp8e4 with perf_mode=DoubleRow over two of the
    three contraction chunks (weights scaled 2^6 each on the host to clear
    the fp8 denormal floor; the 2^12 is divided out in the exp scale).
    They produce transposed activations Qt/Kt [e, t] so the score matmul
    contracts head dims on partitions; V stays bf16 [t, e] (fp8 V or output
    projections would breach the 2e-2 error gate - measured).
  - Scores are computed transposed, St[j, i] (keys on partitions), in a
    causal-compacted [P, 384] layout per head (the fully masked
    keys-hi x queries-lo quarter is never computed), so exp(St) feeds the
    P@V matmul directly with no on-chip transposes.  The 0/1 triangle mask
    is applied multiplicatively on the otherwise idle GpSimd engine.
  - Softmax denominators come from a ones-column matmul over exp(St) into
    the same PSUM bank as the P@V output (O in cols 0:256, denominators in
    256:512); the matmul itself replicates each head's denominator across
    its 64 partitions (engines cannot partition-broadcast from SBUF).
  - The per-query reciprocal is exp(-ln(den)) on the scalar engine: Ln and
    Exp share one activation table set, where a Reciprocal activation would
    force two ~2.7us ACT_TABLE_LOAD swaps per batch.
  - The batch loop is software-pipelined: batch b's attention output (P@V +
    normalize + output projection, consuming exps that finished during the
    previous step) is emitted before batch b+1's scores, keeping the
    per-engine FIFOs free of cross-engine stalls.  Score matmuls chain
    row-group concurrency and P@V/denominator matmuls col-group concurrency
    on the PE array, so the two phases are never interleaved.
  - Group g+1's 12 Q/K projection tiles are emitted 4-per-batch interleaved
    after each of group g's per-batch score phases (group 0 up front), so
    the projection matmuls fill PE bubbles where attention chains wait on
    the scalar engine, instead of sitting head-of-line blocked on their own
    PSUM drains in an up-front block (the PE FIFO runs matmuls strictly in
    emission order).
  - PSUM budget is exactly 8 banks: 3 io (QK proj / V proj / out proj
    rotation), 3 score tiles, 2 P@V banks.  Hard-won tuning notes: any
    bufs=1 structure (or moving banks between these pools) serializes a
    pipeline joint, starves the PE, and re-triggers HAM clock oscillation
    (the PE drops to 1.2 GHz in 6.8us windows - measured +45% wall); the
    scalar engine cannot absorb any PSUM drains on top of its exp/recip
    load; custom-DVE/ISA ops (reciprocal_approx_fast, tensor_mask_reduce)
    fail this container's walrus, and the plain DVE RECIPROCAL runs at
    ~6 cyc/elem - hence the Ln/Exp reciprocal, with Ln running in place
    over the dead denominator region in PSUM (scalar's faster port).
"""

import numpy as np

import concourse.bass as bass
import concourse.tile as tile
from concourse import mybir
from concourse.bass_utils import run_bass_kernel_spmd

P = 128
B, T, C = 256, 256, 384
H, D = 6, 64
NCORES = 8
BL = B // NCORES  # 32 batches per core
G = 4  # batch group for Q/K projection weight reuse
F32 = mybir.dt.float32
F32R = mybir.dt.float32r
BF16 = mybir.dt.bfloat16
F8 = mybir.dt.float8e4
# Q/K weights are scaled by 2^6 each on the host so their ~0.02-sigma values
# clear the fp8e4 denormal floor; the combined 2^12 is divided back out in
# the exp activation's scale argument.
QK_SCALE_LOG2 = 6


def _split_drain_waits(nc, cap=1):
    """This container's walrus rejects instructions carrying more than one
    sync wait ("Too many sync wait commands"); hoist extras onto no-ops
    inserted before (same engine => executed in order)."""
    n_new = 0
    for f in nc.m.functions:
        for bb in f.blocks:
            il = bb.instructions
            out = []
            changed = False
            for inst in list(il):
                si = getattr(inst, "sync_info", None)
                if si is not None and len(si.on_wait) > cap:
                    waits = list(si.on_wait)
                    extra, keep = waits[:-cap], waits[-cap:]
                    for i in range(0, len(extra), cap):
                        nop = mybir.InstNoOp(
                            name=f"I-waitsplit-{n_new}",
                            sync_info=mybir.SyncInfo(
                                on_wait=extra[i : i + cap], on_update=[]
                            ),
                            bass_nofuse=True,
                            engine=inst.engine,
                        )
                        n_new += 1
                        out.append(nop)
                    si.on_wait = keep
                    changed = True
                out.append(inst)
            if changed:
                il.clear()
                il.extend(out)
    return n_new


def build_module(split_waits=True):
    nc = bass.Bass("TRN2", target_bir_lowering=False, debug=False)

    # inputs arrive pre-arranged host-side in the on-chip layout so every
    # DMA is contiguous per partition line (the strided rearrange loads cost
    # ~3x in descriptor overhead)
    xt_d = nc.dram_tensor("xt", [P, 3, BL, T], BF16, kind="ExternalInput").ap()
    xq_d = nc.dram_tensor("xq", [P, 3, BL, T], F8, kind="ExternalInput").ap()
    wq_d = nc.dram_tensor("wq", [P, 3, C], F8, kind="ExternalInput").ap()
    wk_d = nc.dram_tensor("wk", [P, 3, C], F8, kind="ExternalInput").ap()
    wv_d = nc.dram_tensor("wv", [P, 3, C], BF16, kind="ExternalInput").ap()
    wp_d = nc.dram_tensor("wp", [P, 3, C], BF16, kind="ExternalInput").ap()
    bp_d = nc.dram_tensor("bp", [C], F32R, kind="ExternalInput").ap()
    mask_d = nc.dram_tensor("mask", [P, P], BF16, kind="ExternalInput").ap()
    y_d = nc.dram_tensor("y", [BL, T, C], F32, kind="ExternalOutput").ap()

    with tile.TileContext(nc) as tc:
        with (
            tc.tile_pool(name="consts", bufs=1) as consts,
            tc.tile_pool(name="xg", bufs=2) as xg_pool,
            tc.tile_pool(name="xq", bufs=2) as xq_pool,
            tc.tile_pool(name="qt", bufs=4) as qt_pool,
            tc.tile_pool(name="kt", bufs=4) as kt_pool,
            tc.tile_pool(name="vsb", bufs=G + 2) as v_pool,
            tc.tile_pool(name="sts", bufs=8) as sts_pool,
            tc.tile_pool(name="ot", bufs=3) as ot_pool,
            tc.tile_pool(name="ysb", bufs=3) as y_pool,
            tc.tile_pool(name="rsb", bufs=2) as r_pool,
            tc.tile_pool(name="psio", bufs=3, space="PSUM") as ps_io,
            tc.tile_pool(name="psst", bufs=3, space="PSUM") as ps_st,
            tc.tile_pool(name="psorb", bufs=2, space="PSUM") as ps_orb,
        ):
            # ---- constants (compute-critical loads first: the first QK
            # projection only needs x and wq/wk; bias/mask/ones setup is not
            # needed until the first attention phase ~10us later) ----
            xt_r = xt_d
            xq_r = xq_d
            # startup order: the first QK projection needs only wq/wk (fp8,
            # half the bytes) and xq0; everything else arrives while the
            # first projections run
            wq_sb = consts.tile([P, 3, C], F8)
            wk_sb = consts.tile([P, 3, C], F8)
            wv_sb = consts.tile([P, 3, C], BF16)
            wp_sb = consts.tile([P, 3, C], BF16)
            # startup loads ordered by first use: the first Q-projection
            # job needs only wq + the first xq half, so those two
            # descriptors go first (~600ns each on the sync queue)
            xq0 = xq_pool.tile([P, 3, G, T], F8, tag="xq")
            xg0 = xg_pool.tile([P, 3, G, T], BF16, tag="xg")
            h = G // 2
            nc.sync.dma_start(wq_sb[:], wq_d[:])
            nc.sync.dma_start(xq0[:, :, 0:h, :], xq_r[:, :, 0:h, :])
            nc.sync.dma_start(wk_sb[:], wk_d[:])
            nc.sync.dma_start(xq0[:, :, h:G, :], xq_r[:, :, h:G, :])
            nc.sync.dma_start(xg0[:, :, 0:h, :], xt_r[:, :, 0:h, :])
            nc.sync.dma_start(xg0[:, :, h:G, :], xt_r[:, :, h:G, :])
            nc.sync.dma_start(wv_sb[:], wv_d[:])
            nc.sync.dma_start(wp_sb[:], wp_d[:])
            # (Note: a HAM-warmup matmul chain during the startup DMA wait
            # was tried and measured net-negative - the cold-clock warm
            # chain delays the first real projection by about what the
            # clock ramp costs, and mistiming it is expensive.)

            # partition-replication is done with rank-1 matmuls (ones ⊗ row):
            # step-0 partition-broadcast DMAs produce garbage on hardware.
            ones_row = consts.tile([1, P], F32)
            nc.vector.memset(ones_row[:], 1.0)
            ones_row_r = consts.tile([1, P], F32R)
            nc.scalar.activation(
                ones_row_r[:], ones_row[:], mybir.ActivationFunctionType.Copy
            )
            bp_row = consts.tile([1, C], F32R)
            nc.sync.dma_start(bp_row[:], bp_d[None, :])
            bp_sb = consts.tile([P, C], F32)
            mask_sb = consts.tile([P, P], BF16)
            nc.sync.dma_start(mask_sb[:], mask_d[:])

            ones_mat = consts.tile([P, P], BF16)
            nc.vector.memset(ones_mat[:], 1.0)

            def scores_pair(hp, qt, kt, st_pairs):
                """Stage 1, one head pair: score matmuls + exp + causal mask.

                Causal-compacted layout: st_pair [P, h, 384] where free cols
                0:256 hold (keys jt0) x (queries 0:256) and cols 256:384 hold
                (keys jt1) x (queries 128:256).  The fully masked
                (jt1, i<128) quarter is never computed.

                Per-head PSUM tiles with bufs=2 keep two heads' score
                matmuls in flight while the previous head's exp drains.
                """
                st_pair = sts_pool.tile([P, 2, 384], BF16, tag="stp")
                for hidx in range(2):
                    h = 2 * hp + hidx
                    co, half = h // 2, h % 2
                    st_ps = ps_st.tile([P, 384], F32, tag="st")
                    nc.tensor.matmul(
                        st_ps[:, 0:T],
                        (kt[64 * half : 64 * half + 64, co, 0:P]),
                        (qt[64 * half : 64 * half + 64, co, :]),
                        start=True,
                        stop=True,
                    )
                    nc.tensor.matmul(
                        st_ps[:, T : T + P],
                        (kt[64 * half : 64 * half + 64, co, P : 2 * P]),
                        (qt[64 * half : 64 * half + 64, co, P : 2 * P]),
                        start=True,
                        stop=True,
                    )
                    # exp the whole tile unmasked (scores are bounded, so
                    # exp never overflows); the causal mask is applied
                    # multiplicatively below.  scale undoes the 2^12 fp8
                    # weight scaling baked into qt/kt.
                    nc.scalar.activation(
                        st_pair[:, hidx, :],
                        st_ps[:],
                        mybir.ActivationFunctionType.Exp,
                        scale=2.0 ** (-2 * QK_SCALE_LOG2),
                    )
                    # cols 0:128 (jt0, i<128) and 256:384 (jt1, i>=128) are
                    # the same [128,128] 0/1 triangle.  Masked per head (not
                    # per pair) so each mask starts right after its own exp:
                    # the pair's last mask finishes ~530ns earlier, which is
                    # what gates the next pipeline step's P@V.
                    diag_s = st_pair.rearrange("p h (a c) -> p h a c", c=P)[
                        :, hidx : hidx + 1, 0::2, :
                    ]
                    nc.gpsimd.tensor_mul(
                        diag_s,
                        diag_s,
                        mask_sb[:, None, None, :].to_broadcast((P, 1, 2, P)),
                    )
                st_pairs.append(st_pair)

            def attn_pair(hp, st_pair, v_sb, ot):
                """Stage 2, one head pair: P@V and ones-matmul denominators
                (replicated across each head's 64 partitions) into one
                shared PSUM bank (O in cols 0:256, denominators in
                256:512), then the per-pair Ln/Exp reciprocal chain (the
                two share one ACT table set) and the normalize multiply.
                Emission alternates col groups for immediate dual-chain
                feed."""
                h0, h1 = 2 * hp, 2 * hp + 1
                orb = ps_orb.tile([P, 512], F32, tag="orb")
                # even head -> partitions 0:64, odd head -> 64:128 (bf16
                # col tile_position).  Queries below 128 only see jt0 keys,
                # so the jt1 matmul covers N=128 (cols 128:256).
                for jt in range(2):
                    for idx, h in enumerate((h0, h1)):
                        base = 64 * idx
                        if jt == 0:
                            nc.tensor.matmul(
                                orb[base : base + 64, 0:T],
                                (v_sb[:, 0, h, :]),
                                (st_pair[:, idx, 0:T]),
                                start=True,
                                stop=False,
                                tile_position=(0, base),
                            )
                        else:
                            nc.tensor.matmul(
                                orb[base : base + 64, P:T],
                                (v_sb[:, 1, h, :]),
                                (st_pair[:, idx, T : T + P]),
                                start=False,
                                stop=True,
                                tile_position=(0, base),
                            )
                for jt in range(2):
                    for idx in range(2):
                        base = 64 * idx
                        if jt == 0:
                            nc.tensor.matmul(
                                orb[base : base + 64, T : 2 * T],
                                (ones_mat[:, base : base + 64]),
                                (st_pair[:, idx, 0:T]),
                                start=True,
                                stop=False,
                                tile_position=(0, base),
                            )
                        else:
                            nc.tensor.matmul(
                                orb[base : base + 64, T + P : 2 * T],
                                (ones_mat[:, base : base + 64]),
                                (st_pair[:, idx, T : T + P]),
                                start=False,
                                stop=True,
                                tile_position=(0, base),
                            )
                # Ln in place over the denominator region (the dens are
                # dead after this), then Exp(-x) reading PSUM: the scalar
                # engine's PSUM ports are faster than SBUF (errata: SBUF-src
                # 224+FD cyc vs PSUM-src 172+FD), and elementwise in-place
                # is pipeline-safe (out[i] writes ~8 cycles after in[i] is
                # read).
                nc.scalar.activation(
                    orb[:, T : 2 * T],
                    orb[:, T : 2 * T],
                    mybir.ActivationFunctionType.Ln,
                )
                rb = r_pool.tile([P, T], F32, tag="rb")
                nc.scalar.activation(
                    rb[:],
                    orb[:, T : 2 * T],
                    mybir.ActivationFunctionType.Exp,
                    scale=-1.0,
                )
                nc.vector.tensor_mul(ot[:, hp, :], orb[:, 0:T], rb[:])

            def out_proj(b, ot):
                """Stage 2 tail: output projection + bias + store."""
                for tt in range(2):
                    ps = ps_io.tile([P, C], F32, tag="io")
                    for co in range(3):
                        nc.tensor.matmul(
                            ps[:],
                            (ot[:, co, tt * P : (tt + 1) * P]),
                            (wp_sb[:, co, :]),
                            start=(co == 0),
                            stop=(co == 2),
                        )
                    y_sb = y_pool.tile([P, C], F32)
                    nc.vector.tensor_add(y_sb[:], ps[:], bp_sb[:])
                    nc.sync.dma_start(y_d[b, tt * P : (tt + 1) * P, :], y_sb[:])

            # Software pipeline: batch b's scores (tensor) run while batch
            # b-1's exp results feed its P@V (so the tensor engine never
            # waits on the scalar engine's exps), interleaved pair-by-pair.
            prev = None  # (b, st_pairs, v_sb)
            def qk_tiles_alloc(g):
                qt2s = [
                    qt_pool.tile([P, 3, 2, T], BF16, tag="qtb", name=f"qtb_{g}_{bp2}")
                    for bp2 in range(G // 2)
                ]
                kt2s = [
                    kt_pool.tile([P, 3, 2, T], BF16, tag="ktb", name=f"ktb_{g}_{bp2}")
                    for bp2 in range(G // 2)
                ]
                return qt2s, kt2s

            def qk_tile_jobs(xq_g, qt2s, kt2s):
                """One emission closure per Q/K projection output tile
                (proj, eo, bp2).  fp8: contraction chunks co 0..1 in one
                DoubleRow matmul (virtual 256-row array), chunk 2 as a plain
                fp8 matmul, then a drain PSUM->SBUF bf16."""

                def job(w_sb, dst_list, eo, bp2):
                    ps = ps_io.tile([P, 512], F32, tag="io")
                    rhs = xq_g[:, :, 2 * bp2 : 2 * bp2 + 2, :].rearrange(
                        "p c b t -> p c (b t)"
                    )
                    nc.tensor.matmul(
                        ps[:],
                        (w_sb[:, 0:2, eo * P : (eo + 1) * P]),
                        (rhs[:, 0:2, :]),
                        start=True,
                        stop=False,
                        perf_mode=mybir.MatmulPerfMode.DoubleRow,
                    )
                    nc.tensor.matmul(
                        ps[:],
                        (w_sb[:, 2, eo * P : (eo + 1) * P]),
                        (rhs[:, 2, :]),
                        start=False,
                        stop=True,
                    )
                    dst_ap = dst_list[bp2][:, eo, :, :].rearrange("p b t -> p (b t)")
                    # all drains on vector: the scalar engine is saturated
                    # by exps + the reciprocal chain (measured: giving it
                    # even 1-in-3 drains pushes it to 212us busy and the
                    # resulting exp delays re-trigger HAM oscillation)
                    nc.vector.tensor_copy(dst_ap, ps[:])

                from functools import partial

                return [
                    partial(job, w_sb, dst_list, eo, bp2)
                    for bp2 in range(G // 2)
                    for w_sb, dst_list in ((wq_sb, qt2s), (wk_sb, kt2s))
                    for eo in range(3)
                ]

            # Group 0's Q/K tiles are computed up front (startup); from then
            # on group g+1's 12 projection tiles are emitted 4-per-batch
            # interleaved AFTER each batch's score matmuls, so they fill the
            # PE bubbles where score/attention chains wait on the scalar
            # engine's exps - instead of v1's up-front block, whose matmuls
            # sat head-of-line blocked on their own PSUM drains (the PE FIFO
            # executes matmuls strictly in emission order).
            cur_x = (xg0, xq0)
            cur_qk = qk_tiles_alloc(0)
            for j in qk_tile_jobs(xq0, *cur_qk):
                j()
            nxt_x = None
            nxt_qk = None
            jobs = []
            for g in range(BL // G):
                xg, xq = cur_x
                qt2s, kt2s = cur_qk

                if g == 0:
                    # bias broadcast, deferred so the first QK projections
                    # are not stuck behind the bp DMA chain at startup
                    bp_ps = ps_io.tile([P, C], F32, tag="io")
                    nc.tensor.matmul(
                        bp_ps[:], ones_row_r[0:1, :], bp_row[0:1, :],
                        start=True, stop=True,
                    )
                    nc.vector.tensor_copy(bp_sb[:], bp_ps[:])

                for lb in range(G):
                    b = g * G + lb
                    if lb == 0 and g + 1 < BL // G:
                        # prefetch next group's x now; its Q/K projection
                        # jobs start at lb=1, a full batch (~7us) after the
                        # DMA is issued
                        gn = (g + 1) * G
                        h = G // 2
                        xqn = xq_pool.tile([P, 3, G, T], F8, tag="xq", name=f"xq_{g+1}")
                        nc.sync.dma_start(xqn[:, :, 0:h, :], xq_r[:, :, gn : gn + h, :])
                        nc.sync.dma_start(xqn[:, :, h:G, :], xq_r[:, :, gn + h : gn + G, :])
                        xgn = xg_pool.tile([P, 3, G, T], BF16, tag="xg", name=f"xg_{g+1}")
                        nc.sync.dma_start(xgn[:, :, 0:h, :], xt_r[:, :, gn : gn + h, :])
                        nc.sync.dma_start(xgn[:, :, h:G, :], xt_r[:, :, gn + h : gn + G, :])
                        nxt_x = (xgn, xqn)
                        nxt_qk = qk_tiles_alloc(g + 1)
                        jobs = qk_tile_jobs(xqn, *nxt_qk)
                    qt = qt2s[lb // 2][:, :, lb % 2, :]
                    kt = kt2s[lb // 2][:, :, lb % 2, :]

                    # ---- V projection: V[t, e] (x stationary) ----
                    v_sb = v_pool.tile([P, 2, H, D], BF16)
                    for tt in range(2):
                        ps = ps_io.tile([P, C], F32, tag="io")
                        for co in range(3):
                            nc.tensor.matmul(
                                ps[:],
                                (xg[:, co, lb, tt * P : (tt + 1) * P]),
                                (wv_sb[:, co, :]),
                                start=(co == 0),
                                stop=(co == 2),
                            )
                        nc.vector.tensor_copy(
                            v_sb[:, tt, :, :].rearrange("p h d -> p (h d)"), ps[:]
                        )

                    # ---- attention-out[b-1], then scores[b] ----
                    # Phase-grouped: the 12 score matmuls chain row-group
                    # concurrency and the 12 P@V/denominator matmuls chain
                    # col-group concurrency; a score and a P@V matmul can
                    # never overlap on the PE array (scores use all 128
                    # columns, P@V all 128 rows), so mixing them costs
                    # serialization joints.  Batch-level pipelining: batch
                    # b-1's attention output (whose exps finished during the
                    # previous step) runs first; batch b's score exps queue
                    # up behind it and complete during this step's tensor
                    # work.
                    if prev is not None:
                        ot_prev = ot_pool.tile([P, 3, T], BF16, tag="ot", name="ot_prev")
                        for hp in range(3):
                            attn_pair(hp, prev[1][hp], prev[2], ot_prev)
                        out_proj(prev[0], ot_prev)
                    st_pairs = []
                    for hp in range(3):
                        scores_pair(hp, qt, kt, st_pairs)
                    prev = (b, st_pairs, v_sb)
                    # next group's Q/K projection tiles, 4 per batch at
                    # lb=1..3, emitted after this batch's scores.  The even
                    # 4/4/4 spread measured best: 6/6/0 saturates the vector
                    # drain queue mid-group (260.2us), and a half-group-
                    # shifted 3/3/3/3 window regressed ~6us clock-normalized
                    # (the bp2=0 drains land during the next group's first
                    # step and delay its scores).
                    if lb >= 1 and jobs:
                        for j in jobs[4 * (lb - 1) : 4 * lb]:
                            j()
                cur_x = nxt_x
                cur_qk = nxt_qk
                jobs = []

            # ---- drain the last batch ----
            ot_prev = ot_pool.tile([P, 3, T], BF16, tag="ot", name="ot_last")
            for hp in range(3):
                attn_pair(hp, prev[1][hp], prev[2], ot_prev)
            out_proj(prev[0], ot_prev)

    if split_waits:
        _split_drain_waits(nc)
    return nc


_NC = None


def _get_nc():
    global _NC
    if _NC is None:
        _NC = build_module()
    return _NC


def make_mask():
    # [128, 128] 0/1 triangle block: key p visible to query i when p <= i
    import ml_dtypes

    j = np.arange(P)[:, None]
    i = np.arange(P)[None, :]
    return np.where(j <= i, 1.0, 0.0).astype(ml_dtypes.bfloat16)


def prepare_in_maps(x, Wk, Wq, Wv, Wp, bp):
    import ml_dtypes

    bf16 = ml_dtypes.bfloat16
    f8 = ml_dtypes.float8_e4m3
    xf = np.asarray(x, dtype=np.float32).transpose(2, 0, 1)
    xf = xf.reshape(3, P, *xf.shape[1:]).transpose(1, 0, 2, 3)  # [ci, co, b, t]
    xt = np.ascontiguousarray(xf.astype(bf16))
    xq = np.ascontiguousarray(xf.astype(f8))
    # 1/sqrt(D) folded into Wq (exact exponent shift); both Q/K weights are
    # scaled by 2^6 so their values clear the fp8e4 denormal floor (the 2^12
    # total is divided back out in the kernel's exp scale argument)
    s8 = float(2**QK_SCALE_LOG2)

    def chipw(w):  # [C, C] -> [ci, co, e] contiguous
        return np.ascontiguousarray(w.reshape(3, P, C).transpose(1, 0, 2))

    wq = chipw((np.asarray(Wq, dtype=np.float32).T * (0.125 * s8)).astype(f8))
    wk = chipw((np.asarray(Wk, dtype=np.float32).T * s8).astype(f8))
    wv = chipw(np.asarray(Wv, dtype=np.float32).T.astype(bf16))
    wp = chipw(np.asarray(Wp, dtype=np.float32).T.astype(bf16))
    bp = np.asarray(bp, dtype=np.float32)
    mask = make_mask()
    in_maps = []
    for c in range(NCORES):
        in_maps.append(
            {
                "xt": np.ascontiguousarray(xt[:, :, c * BL : (c + 1) * BL, :]),
                "xq": np.ascontiguousarray(xq[:, :, c * BL : (c + 1) * BL, :]),
                "wq": wq,
                "wk": wk,
                "wv": wv,
                "wp": wp,
                "bp": bp,
                "mask": mask,
            }
        )
    return in_maps


def kernel(x, Wk, Wq, Wv, Wp, bp):
    nc = _get_nc()
    in_maps = prepare_in_maps(x, Wk, Wq, Wv, Wp, bp)
    res = run_bass_kernel_spmd(nc, in_maps, list(range(NCORES)))
    return np.concatenate([r["y"] for r in res.results], axis=0)

